# revision 22
# baseline (speedup 1.0000x reference)
"""CommNet GNN message-passing kernel for 8 Trainium2 NeuronCores.

Computation (matches the jax reference):
    h = relu(x @ enc_w1 + enc_b1) @ enc_w2 + enc_b2
    for r in range(R):
        msg[i] = mean over edges (src==i) of h[dst]
        h = h + relu(msg @ comm_w[r] + comm_b[r])
    out = relu(h @ dec_w1 + dec_b1) @ dec_w2 + dec_b2

Sharding: nodes (and their outgoing edges, partitioned by source node id)
are sharded across the 8 cores. MLP weights are replicated. Each comm
round the per-core h shards are AllGather'd into a full copy of h in each
core's DRAM; the per-edge h[dst] reads are then serviced by dma_gather
(512B-row gathers at full DMA bandwidth).

Per-core compute layout ("T layout": features on partitions, nodes on the
free axis) so that every MLP matmul chains without transposes:
  - x is fed pre-transposed by the host as xT [128, NL]
  - segment-mean: per 128-src-node tile, gathered edge rows [edge, feat]
    are reduced into msgT [feat, node] with a one-hot matmul built on the
    fly by a single DVE is_equal op; PSUM accumulates across edge blocks;
    a winv (1/degree) columnwise multiply turns sums into means.
  - only the h rows written for the gather (and nothing else) are
    transposed back to [node, feat], via PE transpose-mode.
Output is returned by the host transposing the per-core outT tiles.
"""

import numpy as np

N = 50000
D = 128
R = 2
NCORES = 8
TILES = 49                # src-node tiles of 128 per core
NL = TILES * 128          # 6272 nodes per core
NP = NCORES * NL          # 50176 padded node count
NSPLIT = 4                # dst-range splits == SWDGE queues; each gather call
                          # runs on its own Q7 core pair and they overlap
QSIZE = NP // NSPLIT      # 12544 rows per split (int16-safe index range)

_PROGRAM_CACHE: dict = {}

# set kernel.TRACE = True (e.g. from test.py) to capture an NTFF profile;
# the BassKernelResults of the last run is stashed in LAST_RESULTS.
TRACE = False
LAST_RESULTS = None


# ----------------------------------------------------------------------------
# Device program
# ----------------------------------------------------------------------------

def build_program(n_cores, tiles, b_q, m_q, n_rounds=R, d=D,
                  np_pad_override=None, use_collectives=True, shared_h=None):
    """Build the SPMD Bass program. Returns the compiled Bacc object.

    b_q: per-split idx-buffer block capacity (len NSPLIT, global max).
    m_q: per-split per-tile gather counts, m_q[q][t] (cross-core max).
    """
    import concourse.bass as bass
    import concourse.bacc as bacc
    import concourse.mybir as mybir
    import concourse.tile as tile

    dt = mybir.dt
    gdt = dt.bfloat16     # gather-path dtype: h_full/gbuf/onehot
    nl = tiles * d
    np_pad = np_pad_override or (n_cores * nl)
    nsp = len(b_q)
    qsz = np_pad // nsp
    # per-tile block layout: split q's blocks start at boff[t][q]
    bl_q = [[(m_q[q][t] + d - 1) // d for q in range(nsp)]
            for t in range(tiles)]
    B_t = [sum(bl_q[t]) for t in range(tiles)]
    B = max(B_t)

    nc = bacc.Bacc("TRN2", target_bir_lowering=False, debug=False,
                   num_devices=n_cores, num_swdge_queues=nsp)

    # -------- kernel I/O --------
    xT_dram = nc.dram_tensor("xT", [d, nl], dt.float32, kind="ExternalInput")
    iq_dram = [nc.dram_tensor(f"iq{q}", [tiles, 128, b_q[q] * 8], dt.int16,
                              kind="ExternalInput") for q in range(nsp)]
    srcv_dram = nc.dram_tensor("srcv", [tiles, 128, B], gdt,
                               kind="ExternalInput")
    winv_dram = nc.dram_tensor("winv", [d, nl], dt.float32, kind="ExternalInput")
    iota_dram = nc.dram_tensor("iota", [d, d], gdt, kind="ExternalInput")
    ident_dram = nc.dram_tensor("ident", [d, d], dt.float32, kind="ExternalInput")
    w_drams = {}
    for wname in ("enc_w1", "enc_w2", "dec_w1", "dec_w2"):
        w_drams[wname] = nc.dram_tensor(wname, [d, d], dt.float32,
                                        kind="ExternalInput")
    for bname in ("enc_b1", "enc_b2", "dec_b1", "dec_b2"):
        w_drams[bname] = nc.dram_tensor(bname, [d, 1], dt.float32,
                                        kind="ExternalInput")
    for r in range(n_rounds):
        w_drams[f"cw{r}"] = nc.dram_tensor(f"cw{r}", [d, d], dt.float32,
                                           kind="ExternalInput")
        w_drams[f"cb{r}"] = nc.dram_tensor(f"cb{r}", [d, 1], dt.float32,
                                           kind="ExternalInput")
    outT_dram = nc.dram_tensor("outT", [d, nl], dt.float32,
                               kind="ExternalOutput")

    Relu = mybir.ActivationFunctionType.Relu
    Ident = mybir.ActivationFunctionType.Identity
    EQ = mybir.AluOpType.is_equal
    MUL = mybir.AluOpType.mult
    ADD = mybir.AluOpType.add

    with tile.TileContext(nc) as tc:
        with (
            tc.tile_pool(name="persist", bufs=1) as pp,
            tc.tile_pool(name="work", bufs=3) as wp,
            tc.tile_pool(name="gather", bufs=4) as gp,
            tc.tile_pool(name="psum", bufs=3, space="PSUM") as ps,
            tc.tile_pool(name="psum2", bufs=2, space="PSUM") as ps2,
            tc.tile_pool(name="dram", bufs=1, space="DRAM") as dp,
        ):
            # ---- persistent SBUF state ----
            xT = pp.tile([d, nl], dt.float32)
            hT = pp.tile([d, nl], dt.float32)
            winv = pp.tile([d, nl], dt.float32)
            iota = pp.tile([d, d], gdt)
            ident = pp.tile([d, d], dt.float32)
            wt = {}
            for wname in ("enc_w1", "enc_w2", "dec_w1", "dec_w2"):
                wt[wname] = pp.tile([d, d], dt.float32, tag=wname, name=wname)
            for bname in ("enc_b1", "enc_b2", "dec_b1", "dec_b2"):
                wt[bname] = pp.tile([d, 1], dt.float32, tag=bname, name=bname)
            for r in range(n_rounds):
                wt[f"cw{r}"] = pp.tile([d, d], dt.float32, tag=f"cw{r}", name=f"cw{r}")
                wt[f"cb{r}"] = pp.tile([d, 1], dt.float32, tag=f"cb{r}", name=f"cb{r}")

            # xT loaded in encoder-group chunks so encoding starts early
            EGx = 4 if tiles % 4 == 0 else (2 if tiles % 2 == 0 else 1)
            for tg in range(0, tiles, EGx):
                gslx = slice(tg * d, (tg + EGx) * d)
                nc.sync.dma_start(xT[:, gslx], xT_dram[:, gslx])
            nc.sync.dma_start(winv[:], winv_dram[:])
            nc.sync.dma_start(iota[:], iota_dram[:])
            nc.sync.dma_start(ident[:], ident_dram[:])
            for k, t in wt.items():
                nc.sync.dma_start(t[:], w_drams[k][:])

            # ---- DRAM scratch ----
            ag_in = dp.tile([nl, d], gdt)
            # Shared-output AllGather is the fast path but needs >4 ranks
            if shared_h is None:
                shared_h = n_cores > 4
            h_addr_space = "Shared" if shared_h else "Local"
            h_full = [dp.tile([np_pad, d], gdt, addr_space=h_addr_space,
                              tag=f"h_full{r}", name=f"h_full{r}")
                      for r in range(n_rounds)]

            def write_h_rows(t):
                """transpose hT[:, tile t] -> [node, feat] bf16 rows -> ag_in."""
                tsl = slice(t * d, (t + 1) * d)
                psT = ps2.tile([d, d], dt.float32, tag="psT")
                nc.tensor.transpose(psT[:], hT[:, tsl], ident[:])
                rowt = wp.tile([d, d], gdt, tag="rowt")
                nc.scalar.copy(rowt[:], psT[:])
                nc.sync.dma_start(ag_in[tsl, :], rowt[:])

            # ================= encoder (4 tiles per matmul, N=512) ======
            EG = 4 if tiles % 4 == 0 else (2 if tiles % 2 == 0 else 1)
            for tg in range(0, tiles, EG):
                gsl = slice(tg * d, (tg + EG) * d)
                psA = ps.tile([d, EG * d], dt.float32, tag="psA",
                              padded_shape=[d, EG * d])
                nc.tensor.matmul(psA[:], wt["enc_w1"][:], xT[:, gsl],
                                 start=True, stop=True)
                h1T = wp.tile([d, EG * d], dt.float32, tag="h1T")
                nc.scalar.activation(h1T[:], psA[:], Relu, bias=wt["enc_b1"][:])
                psB = ps.tile([d, EG * d], dt.float32, tag="psB",
                              padded_shape=[d, EG * d])
                nc.tensor.matmul(psB[:], wt["enc_w2"][:], h1T[:],
                                 start=True, stop=True)
                nc.scalar.activation(hT[:, gsl], psB[:], Ident,
                                     bias=wt["enc_b2"][:])
                for t in range(tg, tg + EG):
                    write_h_rows(t)

            # ================= comm rounds =================
            for r in range(n_rounds):
                if use_collectives:
                    nc.gpsimd.collective_compute(
                        "AllGather",
                        mybir.AluOpType.bypass,
                        replica_groups=[list(range(n_cores))],
                        ins=[ag_in.opt()],
                        outs=[h_full[r].opt()],
                    )
                else:
                    # timeline-model variant: stand-in for the AllGather
                    nc.sync.dma_start(h_full[r][0:nl, :], ag_in[:])
                src_q = [h_full[r][q * qsz:(q + 1) * qsz, :]
                         for q in range(nsp)]

                for t in range(tiles):
                    tsl = slice(t * d, (t + 1) * d)
                    # gather h[dst] for this tile's edges; the nsp dst-range
                    # splits go to different SWDGE queues so their Q7
                    # descriptor generation overlaps
                    gbuf = gp.tile([128, B, d], gdt, tag="gbuf")
                    boff = 0
                    for q in range(nsp):
                        m = m_q[q][t]
                        bl = bl_q[t][q]
                        if not m:
                            continue
                        iq = wp.tile([128, b_q[q] * 8], dt.int16,
                                     tag=f"iq{q}")
                        nc.sync.dma_start(iq[:], iq_dram[q][t])
                        ncol = (m + 15) // 16
                        # m is a multiple of 128 (0-idx padded): the gather
                        # writes every slot of its blocks, nothing stale
                        nc.gpsimd.dma_gather(gbuf[:, boff:boff + bl, :],
                                             src_q[q], iq[:, 0:ncol], m, m,
                                             d, single_packet=False,
                                             queue_num=q)
                        boff += bl
                    # one-hot [128, B_t*d]: oh[p, b*d + j] = (srcv[p, b] == j)
                    srcv = wp.tile([128, B], gdt, tag="srcv")
                    nc.sync.dma_start(srcv[:], srcv_dram[t])
                    oh = gp.tile([128, B * d], gdt, tag="oh")
                    in0 = bass.AP(srcv.tensor, srcv.offset,
                                  [srcv.ap[0], [1, B_t[t]], [0, d]])
                    in1 = bass.AP(iota.tensor, iota.offset,
                                  [iota.ap[0], [0, B_t[t]], [1, d]])
                    out_oh = bass.AP(oh.tensor, oh.offset,
                                     [oh.ap[0], [d, B_t[t]], [1, d]])
                    nc.vector.tensor_tensor(out_oh, in0, in1, EQ)
                    # segment sums: psM[f, n] += gathered_b.T @ onehot_b
                    psM = ps.tile([d, d], dt.float32, tag="psA")
                    for b in range(B_t[t]):
                        nc.tensor.matmul(psM[:], gbuf[:, b, :],
                                         oh[:, b * d:(b + 1) * d],
                                         start=(b == 0), stop=(b == B_t[t] - 1))
                    # mean + comm MLP + residual
                    msgT = wp.tile([d, d], dt.float32, tag="msgT")
                    nc.vector.tensor_tensor(msgT[:], psM[:], winv[:, tsl], MUL)
                    psU = ps.tile([d, d], dt.float32, tag="psB")
                    nc.tensor.matmul(psU[:], wt[f"cw{r}"][:], msgT[:],
                                     start=True, stop=True)
                    updT = wp.tile([d, d], dt.float32, tag="updT")
                    nc.scalar.activation(updT[:], psU[:], Relu,
                                         bias=wt[f"cb{r}"][:])
                    nc.vector.tensor_tensor(hT[:, tsl], hT[:, tsl], updT[:], ADD)
                    if r + 1 < n_rounds:
                        write_h_rows(t)
                    else:
                        # final round: decoder fused per tile so it hides
                        # under the remaining gather descriptor generation
                        psD = ps.tile([d, d], dt.float32, tag="psA",
                                      name="psD")
                        nc.tensor.matmul(psD[:], wt["dec_w1"][:], hT[:, tsl],
                                         start=True, stop=True)
                        d1T = wp.tile([d, d], dt.float32, tag="d1T")
                        nc.scalar.activation(d1T[:], psD[:], Relu,
                                             bias=wt["dec_b1"][:])
                        psE = ps.tile([d, d], dt.float32, tag="psB",
                                      name="psE")
                        nc.tensor.matmul(psE[:], wt["dec_w2"][:], d1T[:],
                                         start=True, stop=True)
                        oT = wp.tile([d, d], dt.float32, tag="oT")
                        nc.scalar.activation(oT[:], psE[:], Ident,
                                             bias=wt["dec_b2"][:])
                        nc.sync.dma_start(outT_dram[:, tsl], oT[:])

    nc.compile()
    return nc


# ----------------------------------------------------------------------------
# Host-side preparation
# ----------------------------------------------------------------------------

def _wrap_idx(idx):
    """int16 idx vector -> [128, n/16] layout: pos j -> (j%16, j//16), x8."""
    n = len(idx)
    a = np.zeros((16, n // 16), np.int16)
    a[np.arange(n) % 16, np.arange(n) // 16] = idx
    return np.tile(a, (8, 1))


def host_prep(x, edge_index, n_cores=NCORES, tiles=TILES, nsp=NSPLIT, d=D):
    """Shard + pad inputs; build per-core gather/one-hot side data.

    Returns (per_core_data, b_q, m_q, B).
    """
    nl = tiles * d
    np_pad = n_cores * nl
    qsz = np_pad // nsp
    n_real = x.shape[0]

    src = np.asarray(edge_index[0]).astype(np.int64)
    dst = np.asarray(edge_index[1]).astype(np.int64)

    cnt = np.bincount(src, minlength=np_pad).astype(np.float32)
    winv_full = 1.0 / np.maximum(cnt, 1.0)

    x_pad = np.zeros((np_pad, d), np.float32)
    x_pad[:n_real] = np.asarray(x, np.float32)

    # sort edges once by (tile, dst): tile-major grouping, ascending dst
    # within a tile; dst-split membership is then contiguous per tile
    tile_of_edge = src // d          # global tile id 0..n_cores*tiles-1
    order = np.lexsort((dst, tile_of_edge))
    src_s, dst_s = src[order], dst[order]
    tile_s = tile_of_edge[order]
    q_s = dst_s // qsz               # dst-range split of each edge

    n_tiles_g = n_cores * tiles
    tile_start = np.searchsorted(tile_s, np.arange(n_tiles_g))
    tile_end = np.searchsorted(tile_s, np.arange(n_tiles_g) + 1)
    # per (global tile, split) edge counts
    n_qt = np.zeros((nsp, n_tiles_g), np.int64)
    for g in range(n_tiles_g):
        cnts = np.bincount(q_s[tile_start[g]:tile_end[g]], minlength=nsp)
        n_qt[:, g] = cnts

    # per-tile gather counts = max across cores (SPMD immediates), padded to
    # full 128-blocks with 0-idx entries: every gbuf slot is then written by
    # the gather (no stale tails -> no memzero) at ~2% extra gen cost
    m_q = [[int(np.ceil(max(1, int(n_qt[q, np.arange(n_cores) * tiles
                                        + t].max())) / 128.0) * 128)
            for t in range(tiles)] for q in range(nsp)]
    b_q = [int(np.ceil(max(m_q[q]) / 128)) for q in range(nsp)]
    bl_q = [[(m_q[q][t] + d - 1) // d for q in range(nsp)]
            for t in range(tiles)]
    B_t = [sum(bl_q[t]) for t in range(tiles)]
    B = max(B_t)

    from ml_dtypes import bfloat16

    per_core = []
    for k in range(n_cores):
        iq_all = [np.full((tiles, 128, b_q[q] * 8), -1, np.int16)
                  for q in range(nsp)]
        srcv_all = np.full((tiles, 128, B), -1.0, np.float32)
        for t in range(tiles):
            g = k * tiles + t
            s0, s1 = tile_start[g], tile_end[g]
            qg = q_s[s0:s1]
            slot_src = np.full(B * 128, -1.0, np.float32)
            boff = 0
            for q in range(nsp):
                e_q = np.flatnonzero(qg == q) + s0
                # idx: real edges, 0-pad to the cross-core max, -1 tail
                idx = np.full(b_q[q] * 128, -1, np.int16)
                idx[:m_q[q][t]] = 0
                idx[:len(e_q)] = dst_s[e_q] - q * qsz
                iq_all[q][t] = _wrap_idx(idx)
                # slot s -> partition s%128, block boff + s//128
                slot_src[boff * 128:boff * 128 + len(e_q)] = \
                    (src_s[e_q] - g * d).astype(np.float32)
                boff += bl_q[t][q]
            srcv_all[t] = slot_src.reshape(B, 128).T
        ksl = slice(k * nl, (k + 1) * nl)
        pc = {
            "xT": np.ascontiguousarray(x_pad[ksl].T),
            "srcv": srcv_all.astype(bfloat16),
            "winv": np.ascontiguousarray(
                np.tile(winv_full[ksl][None, :], (d, 1))),
        }
        for q in range(nsp):
            pc[f"iq{q}"] = iq_all[q]
        per_core.append(pc)
    return per_core, b_q, m_q, B


def kernel(x, edge_index, enc_w1, enc_b1, enc_w2, enc_b2,
           comm_w, comm_b, dec_w1, dec_b1, dec_w2, dec_b2):
    from concourse.bass_utils import run_bass_kernel_spmd

    x = np.asarray(x)
    n_real = x.shape[0]
    per_core, b_q, m_q, B = host_prep(x, np.asarray(edge_index))

    key = (NCORES, TILES, tuple(b_q), tuple(map(tuple, m_q)))
    if key not in _PROGRAM_CACHE:
        _PROGRAM_CACHE[key] = build_program(NCORES, TILES, b_q, m_q,
                                            shared_h=True)
    nc = _PROGRAM_CACHE[key]

    from ml_dtypes import bfloat16
    iota_np = np.tile(np.arange(D, dtype=np.float32)[None, :],
                      (D, 1)).astype(bfloat16)
    ident_np = np.eye(D, dtype=np.float32)
    shared = {
        "iota": iota_np,
        "ident": ident_np,
        "enc_w1": np.asarray(enc_w1, np.float32),
        "enc_w2": np.asarray(enc_w2, np.float32),
        "dec_w1": np.asarray(dec_w1, np.float32),
        "dec_w2": np.asarray(dec_w2, np.float32),
        "enc_b1": np.asarray(enc_b1, np.float32).reshape(D, 1),
        "enc_b2": np.asarray(enc_b2, np.float32).reshape(D, 1),
        "dec_b1": np.asarray(dec_b1, np.float32).reshape(D, 1),
        "dec_b2": np.asarray(dec_b2, np.float32).reshape(D, 1),
    }
    for r in range(R):
        shared[f"cw{r}"] = np.asarray(comm_w[r], np.float32)
        shared[f"cb{r}"] = np.asarray(comm_b[r], np.float32).reshape(D, 1)

    in_maps = [{**shared, **pc} for pc in per_core]
    res = run_bass_kernel_spmd(nc, in_maps, core_ids=list(range(NCORES)),
                               trace=TRACE)
    global LAST_RESULTS
    LAST_RESULTS = res

    out = np.empty((NCORES * NL, D), np.float32)
    for k in range(NCORES):
        out[k * NL:(k + 1) * NL] = res.results[k]["outT"].T
    return out[:n_real]



# revision 24
# speedup vs baseline: 1.1720x; 1.1720x over previous
"""CommNet GNN message-passing kernel for 8 Trainium2 NeuronCores.

Computation (matches the jax reference):
    h = relu(x @ enc_w1 + enc_b1) @ enc_w2 + enc_b2
    for r in range(R):
        msg[i] = mean over edges (src==i) of h[dst]
        h = h + relu(msg @ comm_w[r] + comm_b[r])
    out = relu(h @ dec_w1 + dec_b1) @ dec_w2 + dec_b2

Sharding: nodes (and their outgoing edges, partitioned by source node id)
are sharded across the 8 cores. MLP weights are replicated. Each comm
round the per-core h shards are AllGather'd into a full copy of h in each
core's DRAM; the per-edge h[dst] reads are then serviced by dma_gather
(512B-row gathers at full DMA bandwidth).

Per-core compute layout ("T layout": features on partitions, nodes on the
free axis) so that every MLP matmul chains without transposes:
  - x is fed pre-transposed by the host as xT [128, NL]
  - segment-mean: per 128-src-node tile, gathered edge rows [edge, feat]
    are reduced into msgT [feat, node] with a one-hot matmul built on the
    fly by a single DVE is_equal op; PSUM accumulates across edge blocks;
    a winv (1/degree) columnwise multiply turns sums into means.
  - only the h rows written for the gather (and nothing else) are
    transposed back to [node, feat], via PE transpose-mode.
Output is returned by the host transposing the per-core outT tiles.
"""

import numpy as np

N = 50000
D = 128
R = 2
NCORES = 8
TILES = 49                # src-node tiles of 128 per core
NL = TILES * 128          # 6272 nodes per core
NP = NCORES * NL          # 50176 padded node count
NSPLIT = 4                # dst-range splits == SWDGE queues; each gather call
                          # runs on its own Q7 core pair and they overlap
QSIZE = NP // NSPLIT      # 12544 rows per split (int16-safe index range)

_PROGRAM_CACHE: dict = {}

# set kernel.TRACE = True (e.g. from test.py) to capture an NTFF profile;
# the BassKernelResults of the last run is stashed in LAST_RESULTS.
TRACE = False
LAST_RESULTS = None


# ----------------------------------------------------------------------------
# Device program
# ----------------------------------------------------------------------------

def build_program(n_cores, tiles, b_q, m_q, n_rounds=R, d=D,
                  np_pad_override=None, use_collectives=True, shared_h=None):
    """Build the SPMD Bass program. Returns the compiled Bacc object.

    b_q: per-split idx-buffer block capacity (len NSPLIT, global max).
    m_q: per-split per-tile gather counts, m_q[q][t] (cross-core max).
    """
    import concourse.bass as bass
    import concourse.bacc as bacc
    import concourse.mybir as mybir
    import concourse.tile as tile

    dt = mybir.dt
    gdt = dt.bfloat16     # gather-path dtype: h_full/gbuf/onehot
    nl = tiles * d
    np_pad = np_pad_override or (n_cores * nl)
    nsp = len(b_q)
    qsz = np_pad // nsp
    # per-tile block layout: split q's blocks start at boff[t][q]
    bl_q = [[(m_q[q][t] + d - 1) // d for q in range(nsp)]
            for t in range(tiles)]
    B_t = [sum(bl_q[t]) for t in range(tiles)]
    B = max(B_t)

    nc = bacc.Bacc("TRN2", target_bir_lowering=False, debug=False,
                   num_devices=n_cores, num_swdge_queues=nsp)

    # -------- kernel I/O --------
    xT_dram = nc.dram_tensor("xT", [d, nl], dt.float32, kind="ExternalInput")
    iq_dram = [nc.dram_tensor(f"iq{q}", [tiles, 128, b_q[q] * 8], dt.int16,
                              kind="ExternalInput") for q in range(nsp)]
    srcv_dram = nc.dram_tensor("srcv", [tiles, 128, B], gdt,
                               kind="ExternalInput")
    winv_dram = nc.dram_tensor("winv", [d, nl], dt.float32, kind="ExternalInput")
    iota_dram = nc.dram_tensor("iota", [d, d], gdt, kind="ExternalInput")
    ident_dram = nc.dram_tensor("ident", [d, d], dt.float32, kind="ExternalInput")
    w_drams = {}
    for wname in ("enc_w1", "enc_w2", "dec_w1", "dec_w2"):
        w_drams[wname] = nc.dram_tensor(wname, [d, d], dt.float32,
                                        kind="ExternalInput")
    for bname in ("enc_b1", "enc_b2", "dec_b1", "dec_b2"):
        w_drams[bname] = nc.dram_tensor(bname, [d, 1], dt.float32,
                                        kind="ExternalInput")
    for r in range(n_rounds):
        w_drams[f"cw{r}"] = nc.dram_tensor(f"cw{r}", [d, d], dt.float32,
                                           kind="ExternalInput")
        w_drams[f"cb{r}"] = nc.dram_tensor(f"cb{r}", [d, 1], dt.float32,
                                           kind="ExternalInput")
    outT_dram = nc.dram_tensor("outT", [d, nl], dt.float32,
                               kind="ExternalOutput")

    Relu = mybir.ActivationFunctionType.Relu
    Ident = mybir.ActivationFunctionType.Identity
    EQ = mybir.AluOpType.is_equal
    MUL = mybir.AluOpType.mult
    ADD = mybir.AluOpType.add

    with tile.TileContext(nc) as tc:
        with (
            tc.tile_pool(name="persist", bufs=1) as pp,
            tc.tile_pool(name="work", bufs=3) as wp,
            tc.tile_pool(name="gather", bufs=3) as gp,
            tc.tile_pool(name="psum", bufs=2, space="PSUM") as ps,
            tc.tile_pool(name="psum2", bufs=2, space="PSUM") as ps2,
            tc.tile_pool(name="dram", bufs=1, space="DRAM") as dp,
        ):
            # ---- persistent SBUF state ----
            xT = pp.tile([d, nl], dt.float32)
            hT = pp.tile([d, nl], dt.float32)
            winv = pp.tile([d, nl], dt.float32)
            iota = pp.tile([d, d], gdt)
            ident = pp.tile([d, d], dt.float32)
            wt = {}
            for wname in ("enc_w1", "enc_w2", "dec_w1", "dec_w2"):
                wt[wname] = pp.tile([d, d], dt.float32, tag=wname, name=wname)
            for bname in ("enc_b1", "enc_b2", "dec_b1", "dec_b2"):
                wt[bname] = pp.tile([d, 1], dt.float32, tag=bname, name=bname)
            for r in range(n_rounds):
                wt[f"cw{r}"] = pp.tile([d, d], dt.float32, tag=f"cw{r}", name=f"cw{r}")
                wt[f"cb{r}"] = pp.tile([d, 1], dt.float32, tag=f"cb{r}", name=f"cb{r}")

            nc.sync.dma_start(xT[:], xT_dram[:])
            nc.sync.dma_start(winv[:], winv_dram[:])
            nc.sync.dma_start(iota[:], iota_dram[:])
            nc.sync.dma_start(ident[:], ident_dram[:])
            for k, t in wt.items():
                nc.sync.dma_start(t[:], w_drams[k][:])

            # ---- DRAM scratch ----
            ag_in = dp.tile([nl, d], gdt)
            # Shared-output AllGather is the fast path but needs >4 ranks
            if shared_h is None:
                shared_h = n_cores > 4
            h_addr_space = "Shared" if shared_h else "Local"
            h_full = [dp.tile([np_pad, d], gdt, addr_space=h_addr_space,
                              tag=f"h_full{r}", name=f"h_full{r}")
                      for r in range(n_rounds)]

            def write_h_rows(t):
                """transpose hT[:, tile t] -> [node, feat] bf16 rows -> ag_in."""
                tsl = slice(t * d, (t + 1) * d)
                psT = ps2.tile([d, d], dt.float32, tag="psT")
                nc.tensor.transpose(psT[:], hT[:, tsl], ident[:])
                rowt = wp.tile([d, d], gdt, tag="rowt")
                nc.scalar.copy(rowt[:], psT[:])
                nc.sync.dma_start(ag_in[tsl, :], rowt[:])

            # ================= encoder (4 tiles per matmul, N=512) ======
            EG = 4 if tiles % 4 == 0 else (2 if tiles % 2 == 0 else 1)
            for tg in range(0, tiles, EG):
                gsl = slice(tg * d, (tg + EG) * d)
                psA = ps.tile([d, EG * d], dt.float32, tag="psA",
                              padded_shape=[d, EG * d])
                nc.tensor.matmul(psA[:], wt["enc_w1"][:], xT[:, gsl],
                                 start=True, stop=True)
                h1T = wp.tile([d, EG * d], dt.float32, tag="h1T")
                nc.scalar.activation(h1T[:], psA[:], Relu, bias=wt["enc_b1"][:])
                psB = ps.tile([d, EG * d], dt.float32, tag="psB",
                              padded_shape=[d, EG * d])
                nc.tensor.matmul(psB[:], wt["enc_w2"][:], h1T[:],
                                 start=True, stop=True)
                nc.scalar.activation(hT[:, gsl], psB[:], Ident,
                                     bias=wt["enc_b2"][:])
                for t in range(tg, tg + EG):
                    write_h_rows(t)

            # ================= comm rounds =================
            for r in range(n_rounds):
                if use_collectives:
                    nc.gpsimd.collective_compute(
                        "AllGather",
                        mybir.AluOpType.bypass,
                        replica_groups=[list(range(n_cores))],
                        ins=[ag_in.opt()],
                        outs=[h_full[r].opt()],
                    )
                else:
                    # timeline-model variant: stand-in for the AllGather
                    nc.sync.dma_start(h_full[r][0:nl, :], ag_in[:])
                src_q = [h_full[r][q * qsz:(q + 1) * qsz, :]
                         for q in range(nsp)]

                for t in range(tiles):
                    tsl = slice(t * d, (t + 1) * d)
                    # gather h[dst] for this tile's edges; the nsp dst-range
                    # splits go to different SWDGE queues so their Q7
                    # descriptor generation overlaps
                    gbuf = gp.tile([128, B, d], gdt, tag="gbuf")
                    boff = 0
                    for q in range(nsp):
                        m = m_q[q][t]
                        bl = bl_q[t][q]
                        if not m:
                            continue
                        iq = wp.tile([128, b_q[q] * 8], dt.int16,
                                     tag=f"iq{q}")
                        nc.sync.dma_start(iq[:], iq_dram[q][t])
                        ncol = (m + 15) // 16
                        # m is a multiple of 128 (0-idx padded): the gather
                        # writes every slot of its blocks, nothing stale
                        nc.gpsimd.dma_gather(gbuf[:, boff:boff + bl, :],
                                             src_q[q], iq[:, 0:ncol], m, m,
                                             d, single_packet=False,
                                             queue_num=q)
                        boff += bl
                    # one-hot [128, B_t*d]: oh[p, b*d + j] = (srcv[p, b] == j)
                    srcv = wp.tile([128, B], gdt, tag="srcv")
                    nc.sync.dma_start(srcv[:], srcv_dram[t])
                    oh = gp.tile([128, B * d], gdt, tag="oh")
                    in0 = bass.AP(srcv.tensor, srcv.offset,
                                  [srcv.ap[0], [1, B_t[t]], [0, d]])
                    in1 = bass.AP(iota.tensor, iota.offset,
                                  [iota.ap[0], [0, B_t[t]], [1, d]])
                    out_oh = bass.AP(oh.tensor, oh.offset,
                                     [oh.ap[0], [d, B_t[t]], [1, d]])
                    nc.vector.tensor_tensor(out_oh, in0, in1, EQ)
                    # segment sums: psM[f, n] += gathered_b.T @ onehot_b
                    psM = ps.tile([d, d], dt.float32, tag="psA")
                    for b in range(B_t[t]):
                        nc.tensor.matmul(psM[:], gbuf[:, b, :],
                                         oh[:, b * d:(b + 1) * d],
                                         start=(b == 0), stop=(b == B_t[t] - 1))
                    # mean + comm MLP + residual
                    msgT = wp.tile([d, d], dt.float32, tag="msgT")
                    nc.vector.tensor_tensor(msgT[:], psM[:], winv[:, tsl], MUL)
                    psU = ps.tile([d, d], dt.float32, tag="psB")
                    nc.tensor.matmul(psU[:], wt[f"cw{r}"][:], msgT[:],
                                     start=True, stop=True)
                    updT = wp.tile([d, d], dt.float32, tag="updT")
                    nc.scalar.activation(updT[:], psU[:], Relu,
                                         bias=wt[f"cb{r}"][:])
                    nc.vector.tensor_tensor(hT[:, tsl], hT[:, tsl], updT[:], ADD)
                    if r + 1 < n_rounds:
                        write_h_rows(t)
                    else:
                        # final round: decoder fused per tile so it hides
                        # under the remaining gather descriptor generation
                        psD = ps.tile([d, d], dt.float32, tag="psA",
                                      name="psD")
                        nc.tensor.matmul(psD[:], wt["dec_w1"][:], hT[:, tsl],
                                         start=True, stop=True)
                        d1T = wp.tile([d, d], dt.float32, tag="d1T")
                        nc.scalar.activation(d1T[:], psD[:], Relu,
                                             bias=wt["dec_b1"][:])
                        psE = ps.tile([d, d], dt.float32, tag="psB",
                                      name="psE")
                        nc.tensor.matmul(psE[:], wt["dec_w2"][:], d1T[:],
                                         start=True, stop=True)
                        oT = wp.tile([d, d], dt.float32, tag="oT")
                        nc.scalar.activation(oT[:], psE[:], Ident,
                                             bias=wt["dec_b2"][:])
                        nc.sync.dma_start(outT_dram[:, tsl], oT[:])

    nc.compile()
    return nc


# ----------------------------------------------------------------------------
# Host-side preparation
# ----------------------------------------------------------------------------

def _wrap_idx(idx):
    """int16 idx vector -> [128, n/16] layout: pos j -> (j%16, j//16), x8."""
    n = len(idx)
    a = np.zeros((16, n // 16), np.int16)
    a[np.arange(n) % 16, np.arange(n) // 16] = idx
    return np.tile(a, (8, 1))


def host_prep(x, edge_index, n_cores=NCORES, tiles=TILES, nsp=NSPLIT, d=D):
    """Shard + pad inputs; build per-core gather/one-hot side data.

    Returns (per_core_data, b_q, m_q, B).
    """
    nl = tiles * d
    np_pad = n_cores * nl
    qsz = np_pad // nsp
    n_real = x.shape[0]

    src = np.asarray(edge_index[0]).astype(np.int64)
    dst = np.asarray(edge_index[1]).astype(np.int64)

    cnt = np.bincount(src, minlength=np_pad).astype(np.float32)
    winv_full = 1.0 / np.maximum(cnt, 1.0)

    x_pad = np.zeros((np_pad, d), np.float32)
    x_pad[:n_real] = np.asarray(x, np.float32)

    # sort edges once by (tile, dst): tile-major grouping, ascending dst
    # within a tile; dst-split membership is then contiguous per tile
    tile_of_edge = src // d          # global tile id 0..n_cores*tiles-1
    order = np.lexsort((dst, tile_of_edge))
    src_s, dst_s = src[order], dst[order]
    tile_s = tile_of_edge[order]
    q_s = dst_s // qsz               # dst-range split of each edge

    n_tiles_g = n_cores * tiles
    tile_start = np.searchsorted(tile_s, np.arange(n_tiles_g))
    tile_end = np.searchsorted(tile_s, np.arange(n_tiles_g) + 1)
    # per (global tile, split) edge counts
    n_qt = np.zeros((nsp, n_tiles_g), np.int64)
    for g in range(n_tiles_g):
        cnts = np.bincount(q_s[tile_start[g]:tile_end[g]], minlength=nsp)
        n_qt[:, g] = cnts

    # per-tile gather counts = max across cores (SPMD immediates), padded to
    # full 128-blocks with 0-idx entries: every gbuf slot is then written by
    # the gather (no stale tails -> no memzero) at ~2% extra gen cost
    m_q = [[int(np.ceil(max(1, int(n_qt[q, np.arange(n_cores) * tiles
                                        + t].max())) / 128.0) * 128)
            for t in range(tiles)] for q in range(nsp)]
    b_q = [int(np.ceil(max(m_q[q]) / 128)) for q in range(nsp)]
    bl_q = [[(m_q[q][t] + d - 1) // d for q in range(nsp)]
            for t in range(tiles)]
    B_t = [sum(bl_q[t]) for t in range(tiles)]
    B = max(B_t)

    from ml_dtypes import bfloat16

    per_core = []
    for k in range(n_cores):
        iq_all = [np.full((tiles, 128, b_q[q] * 8), -1, np.int16)
                  for q in range(nsp)]
        srcv_all = np.full((tiles, 128, B), -1.0, np.float32)
        for t in range(tiles):
            g = k * tiles + t
            s0, s1 = tile_start[g], tile_end[g]
            qg = q_s[s0:s1]
            slot_src = np.full(B * 128, -1.0, np.float32)
            boff = 0
            for q in range(nsp):
                e_q = np.flatnonzero(qg == q) + s0
                # idx: real edges, 0-pad to the cross-core max, -1 tail
                idx = np.full(b_q[q] * 128, -1, np.int16)
                idx[:m_q[q][t]] = 0
                idx[:len(e_q)] = dst_s[e_q] - q * qsz
                iq_all[q][t] = _wrap_idx(idx)
                # slot s -> partition s%128, block boff + s//128
                slot_src[boff * 128:boff * 128 + len(e_q)] = \
                    (src_s[e_q] - g * d).astype(np.float32)
                boff += bl_q[t][q]
            srcv_all[t] = slot_src.reshape(B, 128).T
        ksl = slice(k * nl, (k + 1) * nl)
        pc = {
            "xT": np.ascontiguousarray(x_pad[ksl].T),
            "srcv": srcv_all.astype(bfloat16),
            "winv": np.ascontiguousarray(
                np.tile(winv_full[ksl][None, :], (d, 1))),
        }
        for q in range(nsp):
            pc[f"iq{q}"] = iq_all[q]
        per_core.append(pc)
    return per_core, b_q, m_q, B


def kernel(x, edge_index, enc_w1, enc_b1, enc_w2, enc_b2,
           comm_w, comm_b, dec_w1, dec_b1, dec_w2, dec_b2):
    from concourse.bass_utils import run_bass_kernel_spmd

    x = np.asarray(x)
    n_real = x.shape[0]
    per_core, b_q, m_q, B = host_prep(x, np.asarray(edge_index))

    key = (NCORES, TILES, tuple(b_q), tuple(map(tuple, m_q)))
    if key not in _PROGRAM_CACHE:
        _PROGRAM_CACHE[key] = build_program(NCORES, TILES, b_q, m_q,
                                            shared_h=True)
    nc = _PROGRAM_CACHE[key]

    from ml_dtypes import bfloat16
    iota_np = np.tile(np.arange(D, dtype=np.float32)[None, :],
                      (D, 1)).astype(bfloat16)
    ident_np = np.eye(D, dtype=np.float32)
    shared = {
        "iota": iota_np,
        "ident": ident_np,
        "enc_w1": np.asarray(enc_w1, np.float32),
        "enc_w2": np.asarray(enc_w2, np.float32),
        "dec_w1": np.asarray(dec_w1, np.float32),
        "dec_w2": np.asarray(dec_w2, np.float32),
        "enc_b1": np.asarray(enc_b1, np.float32).reshape(D, 1),
        "enc_b2": np.asarray(enc_b2, np.float32).reshape(D, 1),
        "dec_b1": np.asarray(dec_b1, np.float32).reshape(D, 1),
        "dec_b2": np.asarray(dec_b2, np.float32).reshape(D, 1),
    }
    for r in range(R):
        shared[f"cw{r}"] = np.asarray(comm_w[r], np.float32)
        shared[f"cb{r}"] = np.asarray(comm_b[r], np.float32).reshape(D, 1)

    in_maps = [{**shared, **pc} for pc in per_core]
    res = run_bass_kernel_spmd(nc, in_maps, core_ids=list(range(NCORES)),
                               trace=TRACE)
    global LAST_RESULTS
    LAST_RESULTS = res

    out = np.empty((NCORES * NL, D), np.float32)
    for k in range(NCORES):
        out[k * NL:(k + 1) * NL] = res.results[k]["outT"].T
    return out[:n_real]



# revision 34
# speedup vs baseline: 1.2221x; 1.0427x over previous
"""CommNet GNN message-passing kernel for 8 Trainium2 NeuronCores.

Computation (matches the jax reference):
    h = relu(x @ enc_w1 + enc_b1) @ enc_w2 + enc_b2
    for r in range(R):
        msg[i] = mean over edges (src==i) of h[dst]
        h = h + relu(msg @ comm_w[r] + comm_b[r])
    out = relu(h @ dec_w1 + dec_b1) @ dec_w2 + dec_b2

Sharding: nodes (and their outgoing edges, partitioned by source node id)
are sharded across the 8 cores. MLP weights are replicated. Each comm
round the per-core h shards are AllGather'd into a full copy of h in each
core's DRAM; the per-edge h[dst] reads are then serviced by dma_gather
(512B-row gathers at full DMA bandwidth).

Per-core compute layout ("T layout": features on partitions, nodes on the
free axis) so that every MLP matmul chains without transposes:
  - x is fed pre-transposed by the host as xT [128, NL]
  - segment-mean: per 128-src-node tile, gathered edge rows [edge, feat]
    are reduced into msgT [feat, node] with a one-hot matmul built on the
    fly by a single DVE is_equal op; PSUM accumulates across edge blocks;
    a winv (1/degree) columnwise multiply turns sums into means.
  - only the h rows written for the gather (and nothing else) are
    transposed back to [node, feat], via PE transpose-mode.
Output is returned by the host transposing the per-core outT tiles.
"""

import numpy as np

N = 50000
D = 128
R = 2
NCORES = 8
TILES = 49                # src-node tiles of 128 per core
NL = TILES * 128          # 6272 nodes per core
NP = NCORES * NL          # 50176 padded node count
NSPLIT = 4                # dst splits == SWDGE queues; each gather call
                          # runs on its own Q7 core pair and they overlap
# tile-group boundaries for the split AllGathers: split g covers every
# rank's local rows [GB[g], GB[g+1]); its AllGather fires as soon as the
# owning tiles are updated, overlapping the rest of the round's compute
GTILES = [13, 13, 13, 10]
GB = [0, 13 * 128, 26 * 128, 39 * 128, 49 * 128]

_PROGRAM_CACHE: dict = {}

# set kernel.TRACE = True (e.g. from test.py) to capture an NTFF profile;
# the BassKernelResults of the last run is stashed in LAST_RESULTS.
TRACE = False
LAST_RESULTS = None


# ----------------------------------------------------------------------------
# Device program
# ----------------------------------------------------------------------------

def build_program(n_cores, tiles, b_q, m_q, n_rounds=R, d=D,
                  np_pad_override=None, use_collectives=True, shared_h=None):
    """Build the SPMD Bass program. Returns the compiled Bacc object.

    b_q: per-split idx-buffer block capacity (len NSPLIT, global max).
    m_q: per-split per-tile gather counts, m_q[q][t] (cross-core max).
    """
    import concourse.bass as bass
    import concourse.bacc as bacc
    import concourse.mybir as mybir
    import concourse.tile as tile

    dt = mybir.dt
    gdt = dt.bfloat16     # gather-path dtype: h_part/gbuf/onehot
    nl = tiles * d
    np_pad = np_pad_override or (n_cores * nl)
    nsp = len(b_q)
    sz_g = [GB[g + 1] - GB[g] for g in range(nsp)]
    # per-tile block layout: split q's blocks start at boff[t][q]
    bl_q = [[(m_q[q][t] + d - 1) // d for q in range(nsp)]
            for t in range(tiles)]
    B_t = [sum(bl_q[t]) for t in range(tiles)]
    B = max(B_t)
    # last tile of each group -> AllGather emission point
    g_end = {}
    acc = 0
    for g, n_t in enumerate(GTILES):
        acc += n_t
        g_end[acc - 1] = g

    nc = bacc.Bacc("TRN2", target_bir_lowering=False, debug=False,
                   num_devices=n_cores, num_swdge_queues=nsp)

    # -------- kernel I/O --------
    xT_dram = nc.dram_tensor("xT", [d, nl], dt.float32, kind="ExternalInput")
    iq_dram = [nc.dram_tensor(f"iq{q}", [tiles, 128, b_q[q] * 8], dt.int16,
                              kind="ExternalInput") for q in range(nsp)]
    srcv_dram = nc.dram_tensor("srcv", [tiles, 128, B], gdt,
                               kind="ExternalInput")
    winv_dram = nc.dram_tensor("winv", [d, nl], dt.float32, kind="ExternalInput")
    iota_dram = nc.dram_tensor("iota", [d, d], gdt, kind="ExternalInput")
    ident_dram = nc.dram_tensor("ident", [d, d], dt.float32, kind="ExternalInput")
    w_drams = {}
    for wname in ("enc_w1", "enc_w2", "dec_w1", "dec_w2"):
        w_drams[wname] = nc.dram_tensor(wname, [d, d], dt.float32,
                                        kind="ExternalInput")
    for bname in ("enc_b1", "enc_b2", "dec_b1", "dec_b2"):
        w_drams[bname] = nc.dram_tensor(bname, [d, 1], dt.float32,
                                        kind="ExternalInput")
    for r in range(n_rounds):
        w_drams[f"cw{r}"] = nc.dram_tensor(f"cw{r}", [d, d], dt.float32,
                                           kind="ExternalInput")
        w_drams[f"cb{r}"] = nc.dram_tensor(f"cb{r}", [d, 1], dt.float32,
                                           kind="ExternalInput")
    outT_dram = nc.dram_tensor("outT", [d, nl], dt.float32,
                               kind="ExternalOutput")

    Relu = mybir.ActivationFunctionType.Relu
    Ident = mybir.ActivationFunctionType.Identity
    EQ = mybir.AluOpType.is_equal
    MUL = mybir.AluOpType.mult
    ADD = mybir.AluOpType.add

    with tile.TileContext(nc) as tc:
        with (
            tc.tile_pool(name="persist", bufs=1) as pp,
            tc.tile_pool(name="work", bufs=3) as wp,
            tc.tile_pool(name="gather", bufs=3) as gp,
            tc.tile_pool(name="psum", bufs=2, space="PSUM") as ps,
            tc.tile_pool(name="psum2", bufs=2, space="PSUM") as ps2,
            tc.tile_pool(name="dram", bufs=1, space="DRAM") as dp,
        ):
            # ---- persistent SBUF state ----
            xT = pp.tile([d, nl], dt.float32)
            hT = pp.tile([d, nl], dt.float32)
            winv = pp.tile([d, nl], dt.float32)
            iota = pp.tile([d, d], gdt)
            ident = pp.tile([d, d], dt.float32)
            wt = {}
            for wname in ("enc_w1", "enc_w2", "dec_w1", "dec_w2"):
                wt[wname] = pp.tile([d, d], dt.float32, tag=wname, name=wname)
            for bname in ("enc_b1", "enc_b2", "dec_b1", "dec_b2"):
                wt[bname] = pp.tile([d, 1], dt.float32, tag=bname, name=bname)
            for r in range(n_rounds):
                wt[f"cw{r}"] = pp.tile([d, d], dt.float32, tag=f"cw{r}", name=f"cw{r}")
                wt[f"cb{r}"] = pp.tile([d, 1], dt.float32, tag=f"cb{r}", name=f"cb{r}")

            nc.sync.dma_start(xT[:], xT_dram[:])
            nc.sync.dma_start(winv[:], winv_dram[:])
            nc.sync.dma_start(iota[:], iota_dram[:])
            nc.sync.dma_start(ident[:], ident_dram[:])
            for k, t in wt.items():
                nc.sync.dma_start(t[:], w_drams[k][:])

            # ---- DRAM scratch ----
            ag_in = dp.tile([nl, d], gdt)
            # Shared-output AllGather is the fast path but needs >4 ranks
            if shared_h is None:
                shared_h = n_cores > 4
            h_addr_space = "Shared" if shared_h else "Local"
            # per-(round, group) gathered-h buffers: rank-major concat of
            # every rank's local rows [GB[g], GB[g+1])
            h_part = [[dp.tile([n_cores * sz_g[g], d], gdt,
                               addr_space=h_addr_space,
                               tag=f"h_part{r}_{g}", name=f"h_part{r}_{g}")
                       for g in range(nsp)] for r in range(n_rounds)]

            def write_h_rows(t):
                """transpose hT[:, tile t] -> [node, feat] bf16 rows -> ag_in."""
                tsl = slice(t * d, (t + 1) * d)
                psT = ps2.tile([d, d], dt.float32, tag="psT")
                nc.tensor.transpose(psT[:], hT[:, tsl], ident[:])
                rowt = wp.tile([d, d], gdt, tag="rowt")
                nc.scalar.copy(rowt[:], psT[:])
                nc.sync.dma_start(ag_in[tsl, :], rowt[:])

            def allgather_group(r, g):
                nc.gpsimd.collective_compute(
                    "AllGather",
                    mybir.AluOpType.bypass,
                    replica_groups=[list(range(n_cores))],
                    ins=[ag_in[GB[g]:GB[g + 1], :].opt()],
                    outs=[h_part[r][g].opt()],
                )

            # ================= encoder (4 tiles per matmul, N=512) ======
            EG = 4 if tiles % 4 == 0 else (2 if tiles % 2 == 0 else 1)
            for tg in range(0, tiles, EG):
                gsl = slice(tg * d, (tg + EG) * d)
                psA = ps.tile([d, EG * d], dt.float32, tag="psA",
                              padded_shape=[d, EG * d])
                nc.tensor.matmul(psA[:], wt["enc_w1"][:], xT[:, gsl],
                                 start=True, stop=True)
                h1T = wp.tile([d, EG * d], dt.float32, tag="h1T")
                nc.scalar.activation(h1T[:], psA[:], Relu, bias=wt["enc_b1"][:])
                psB = ps.tile([d, EG * d], dt.float32, tag="psB",
                              padded_shape=[d, EG * d])
                nc.tensor.matmul(psB[:], wt["enc_w2"][:], h1T[:],
                                 start=True, stop=True)
                nc.scalar.activation(hT[:, gsl], psB[:], Ident,
                                     bias=wt["enc_b2"][:])
                for t in range(tg, tg + EG):
                    write_h_rows(t)
                    if t in g_end:
                        allgather_group(0, g_end[t])

            # ================= comm rounds =================
            for r in range(n_rounds):
                # the group AllGathers for round r were already emitted
                # (encoder for r=0, round r-1's tile loop otherwise); the
                # gather calls below wait on their h_part buffers directly
                src_q = [h_part[r][q][:] for q in range(nsp)]

                for t in range(tiles):
                    tsl = slice(t * d, (t + 1) * d)
                    # gather h[dst] for this tile's edges; the nsp dst-range
                    # splits go to different SWDGE queues so their Q7
                    # descriptor generation overlaps
                    gbuf = gp.tile([128, B, d], gdt, tag="gbuf")
                    boff = 0
                    for q in range(nsp):
                        m = m_q[q][t]
                        bl = bl_q[t][q]
                        if not m:
                            continue
                        iq = wp.tile([128, b_q[q] * 8], dt.int16,
                                     tag=f"iq{q}")
                        nc.sync.dma_start(iq[:], iq_dram[q][t])
                        ncol = (m + 15) // 16
                        if m % 128:
                            # pre-zero the split's last block: the gather
                            # stops at m and the tail would be stale (onehot
                            # is 0 there, but 0*NaN would poison the matmul).
                            # Program-order WAW keeps memzero before gather.
                            nc.scalar.memzero(gbuf[:, boff + bl - 1, :])
                        nc.gpsimd.dma_gather(gbuf[:, boff:boff + bl, :],
                                             src_q[q], iq[:, 0:ncol], m, m,
                                             d, single_packet=False,
                                             queue_num=q)
                        boff += bl
                    # one-hot [128, B_t*d]: oh[p, b*d + j] = (srcv[p, b] == j)
                    srcv = wp.tile([128, B], gdt, tag="srcv")
                    nc.sync.dma_start(srcv[:], srcv_dram[t])
                    oh = gp.tile([128, B * d], gdt, tag="oh")
                    in0 = bass.AP(srcv.tensor, srcv.offset,
                                  [srcv.ap[0], [1, B_t[t]], [0, d]])
                    in1 = bass.AP(iota.tensor, iota.offset,
                                  [iota.ap[0], [0, B_t[t]], [1, d]])
                    out_oh = bass.AP(oh.tensor, oh.offset,
                                     [oh.ap[0], [d, B_t[t]], [1, d]])
                    nc.vector.tensor_tensor(out_oh, in0, in1, EQ)
                    # segment sums: psM[f, n] += gathered_b.T @ onehot_b
                    psM = ps.tile([d, d], dt.float32, tag="psA")
                    for b in range(B_t[t]):
                        nc.tensor.matmul(psM[:], gbuf[:, b, :],
                                         oh[:, b * d:(b + 1) * d],
                                         start=(b == 0), stop=(b == B_t[t] - 1))
                    # mean + comm MLP + residual
                    msgT = wp.tile([d, d], dt.float32, tag="msgT")
                    nc.vector.tensor_tensor(msgT[:], psM[:], winv[:, tsl], MUL)
                    psU = ps.tile([d, d], dt.float32, tag="psB")
                    nc.tensor.matmul(psU[:], wt[f"cw{r}"][:], msgT[:],
                                     start=True, stop=True)
                    updT = wp.tile([d, d], dt.float32, tag="updT")
                    nc.scalar.activation(updT[:], psU[:], Relu,
                                         bias=wt[f"cb{r}"][:])
                    nc.vector.tensor_tensor(hT[:, tsl], hT[:, tsl], updT[:], ADD)
                    if r + 1 < n_rounds:
                        write_h_rows(t)
                        if t in g_end:
                            allgather_group(r + 1, g_end[t])
                    else:
                        # final round: decoder fused per tile so it hides
                        # under the remaining gather descriptor generation
                        psD = ps.tile([d, d], dt.float32, tag="psA",
                                      name="psD")
                        nc.tensor.matmul(psD[:], wt["dec_w1"][:], hT[:, tsl],
                                         start=True, stop=True)
                        d1T = wp.tile([d, d], dt.float32, tag="d1T")
                        nc.scalar.activation(d1T[:], psD[:], Relu,
                                             bias=wt["dec_b1"][:])
                        psE = ps.tile([d, d], dt.float32, tag="psB",
                                      name="psE")
                        nc.tensor.matmul(psE[:], wt["dec_w2"][:], d1T[:],
                                         start=True, stop=True)
                        oT = wp.tile([d, d], dt.float32, tag="oT")
                        nc.scalar.activation(oT[:], psE[:], Ident,
                                             bias=wt["dec_b2"][:])
                        nc.sync.dma_start(outT_dram[:, tsl], oT[:])

    nc.compile()
    return nc


# ----------------------------------------------------------------------------
# Host-side preparation
# ----------------------------------------------------------------------------

def _wrap_idx(idx):
    """int16 idx vector -> [128, n/16] layout: pos j -> (j%16, j//16), x8."""
    n = len(idx)
    a = np.zeros((16, n // 16), np.int16)
    a[np.arange(n) % 16, np.arange(n) // 16] = idx
    return np.tile(a, (8, 1))


def host_prep(x, edge_index, n_cores=NCORES, tiles=TILES, nsp=NSPLIT, d=D):
    """Shard + pad inputs; build per-core gather/one-hot side data.

    Returns (per_core_data, b_q, m_q, B).
    """
    nl = tiles * d
    np_pad = n_cores * nl
    n_real = x.shape[0]

    src = np.asarray(edge_index[0]).astype(np.int64)
    dst = np.asarray(edge_index[1]).astype(np.int64)

    cnt = np.bincount(src, minlength=np_pad).astype(np.float32)
    winv_full = 1.0 / np.maximum(cnt, 1.0)

    x_pad = np.zeros((np_pad, d), np.float32)
    x_pad[:n_real] = np.asarray(x, np.float32)

    # sort edges once by (tile, dst): tile-major grouping, ascending dst
    # within a tile; dst-split membership is then contiguous per tile
    tile_of_edge = src // d          # global tile id 0..n_cores*tiles-1
    order = np.lexsort((dst, tile_of_edge))
    src_s, dst_s = src[order], dst[order]
    tile_s = tile_of_edge[order]
    # split of each edge = local-row group of its dst (h_part layout);
    # buffer row within split g: rank * sz_g + (local - GB[g])
    gb = np.asarray(GB)
    sz_g = gb[1:] - gb[:-1]
    dst_rank = dst_s // nl
    dst_loc = dst_s % nl
    q_s = np.searchsorted(gb, dst_loc, side="right") - 1
    dst_buf = dst_rank * sz_g[q_s] + (dst_loc - gb[q_s])

    n_tiles_g = n_cores * tiles
    tile_start = np.searchsorted(tile_s, np.arange(n_tiles_g))
    tile_end = np.searchsorted(tile_s, np.arange(n_tiles_g) + 1)
    # per (global tile, split) edge counts
    n_qt = np.zeros((nsp, n_tiles_g), np.int64)
    for g in range(n_tiles_g):
        cnts = np.bincount(q_s[tile_start[g]:tile_end[g]], minlength=nsp)
        n_qt[:, g] = cnts

    # per-tile gather counts = max across cores (SPMD immediates); idx
    # buffers sized to the global max, -1 tails are skipped by the ucode
    m_q = [[max(1, int(n_qt[q, np.arange(n_cores) * tiles + t].max()))
            for t in range(tiles)] for q in range(nsp)]
    b_q = [int(np.ceil(max(m_q[q]) / 128)) for q in range(nsp)]
    bl_q = [[(m_q[q][t] + d - 1) // d for q in range(nsp)]
            for t in range(tiles)]
    B_t = [sum(bl_q[t]) for t in range(tiles)]
    B = max(B_t)

    from ml_dtypes import bfloat16

    per_core = []
    for k in range(n_cores):
        iq_all = [np.full((tiles, 128, b_q[q] * 8), -1, np.int16)
                  for q in range(nsp)]
        srcv_all = np.full((tiles, 128, B), -1.0, np.float32)
        for t in range(tiles):
            g = k * tiles + t
            s0, s1 = tile_start[g], tile_end[g]
            qg = q_s[s0:s1]
            slot_src = np.full(B * 128, -1.0, np.float32)
            boff = 0
            for q in range(nsp):
                e_q = np.flatnonzero(qg == q) + s0
                # idx: real edges, 0-pad to the cross-core max, -1 tail
                idx = np.full(b_q[q] * 128, -1, np.int16)
                idx[:m_q[q][t]] = 0
                idx[:len(e_q)] = dst_buf[e_q]
                iq_all[q][t] = _wrap_idx(idx)
                # slot s -> partition s%128, block boff + s//128
                slot_src[boff * 128:boff * 128 + len(e_q)] = \
                    (src_s[e_q] - g * d).astype(np.float32)
                boff += bl_q[t][q]
            srcv_all[t] = slot_src.reshape(B, 128).T
        ksl = slice(k * nl, (k + 1) * nl)
        pc = {
            "xT": np.ascontiguousarray(x_pad[ksl].T),
            "srcv": srcv_all.astype(bfloat16),
            "winv": np.ascontiguousarray(
                np.tile(winv_full[ksl][None, :], (d, 1))),
        }
        for q in range(nsp):
            pc[f"iq{q}"] = iq_all[q]
        per_core.append(pc)
    return per_core, b_q, m_q, B


def kernel(x, edge_index, enc_w1, enc_b1, enc_w2, enc_b2,
           comm_w, comm_b, dec_w1, dec_b1, dec_w2, dec_b2):
    from concourse.bass_utils import run_bass_kernel_spmd

    x = np.asarray(x)
    n_real = x.shape[0]
    per_core, b_q, m_q, B = host_prep(x, np.asarray(edge_index))

    key = (NCORES, TILES, tuple(b_q), tuple(map(tuple, m_q)))
    if key not in _PROGRAM_CACHE:
        _PROGRAM_CACHE[key] = build_program(NCORES, TILES, b_q, m_q,
                                            shared_h=True)
    nc = _PROGRAM_CACHE[key]

    from ml_dtypes import bfloat16
    iota_np = np.tile(np.arange(D, dtype=np.float32)[None, :],
                      (D, 1)).astype(bfloat16)
    ident_np = np.eye(D, dtype=np.float32)
    shared = {
        "iota": iota_np,
        "ident": ident_np,
        "enc_w1": np.asarray(enc_w1, np.float32),
        "enc_w2": np.asarray(enc_w2, np.float32),
        "dec_w1": np.asarray(dec_w1, np.float32),
        "dec_w2": np.asarray(dec_w2, np.float32),
        "enc_b1": np.asarray(enc_b1, np.float32).reshape(D, 1),
        "enc_b2": np.asarray(enc_b2, np.float32).reshape(D, 1),
        "dec_b1": np.asarray(dec_b1, np.float32).reshape(D, 1),
        "dec_b2": np.asarray(dec_b2, np.float32).reshape(D, 1),
    }
    for r in range(R):
        shared[f"cw{r}"] = np.asarray(comm_w[r], np.float32)
        shared[f"cb{r}"] = np.asarray(comm_b[r], np.float32).reshape(D, 1)

    in_maps = [{**shared, **pc} for pc in per_core]
    res = run_bass_kernel_spmd(nc, in_maps, core_ids=list(range(NCORES)),
                               trace=TRACE)
    global LAST_RESULTS
    LAST_RESULTS = res

    out = np.empty((NCORES * NL, D), np.float32)
    for k in range(NCORES):
        out[k * NL:(k + 1) * NL] = res.results[k]["outT"].T
    return out[:n_real]

